# revision 77
# baseline (speedup 1.0000x reference)
"""Trainium2 Bass kernel for MultiHeadAttention + residual + BatchNorm.

Model (reference):
  q = query @ Wq.T ; k = key @ Wk.T ; v = key @ Wv.T    (per-head split)
  score = q k^T / sqrt(D), causal mask, softmax over keys
  res   = (attn @ v) + query
  out   = batchnorm(res over all (N*L) rows, per feature) * gamma + beta

Sharding over 8 cores: FEATURE sharding. Core c owns heads {2c, 2c+1}
(features [128c, 128c+128)) for ALL batches. BatchNorm statistics are
then core-local (sums over all N*L rows of the core's own features), so
no collective is needed at all.

All matmul operands are bf16 (PSUM accumulation stays fp32); the
residual add, batch-norm statistics and outputs are fp32.

Per 512-row query chunk ic and batch n the attention inner loop walks
key blocks jc (128 keys each, causal): PE computes both heads' scores
into one PSUM tile [128 j, 1024 (h,i)], ScalarE applies a single merged
exp -> bf16, DVE masks the diagonal block with a triangular multiply,
and PE accumulates the *flipped* attention-V product out[i, p] with a
ones-column appended to V so softmax denominators fall out of the same
matmuls. Projections for chunk ic+1 are emitted as small PE micro-ops
paced between attention iterations so the scalar engine never starves.
"""

import math
import sys

sys.path.insert(0, "/opt/trn_rl_repo")

import numpy as np
import ml_dtypes

import concourse.bass as bass
import concourse.mybir as mybir
from concourse import bacc
import concourse.tile as tile

F32 = mybir.dt.float32
F32R = mybir.dt.float32r
BF16 = mybir.dt.bfloat16
FP8 = mybir.dt.float8e4
I32 = mybir.dt.int32
BF16_NP = ml_dtypes.bfloat16
FP8_NP = mybir.dt.np(FP8)
# q/k weights are scaled by 16 on the host so fp8e4 stays out of the
# subnormal range; q.k scores come out 256x large, compensated in the
# exp's scale argument
W8_SCALE = 16.0

N = 4
L = 2048
D = 1024
H = 16
P = 64
NCORES = 8
FC = D // NCORES       # features per core = 128
H2 = 2                 # heads per core
EPS = 1e-5
SCALE = 1.0 / math.sqrt(D)
NL = N * L             # 8192 rows in the global batch norm

# Schraudolph approximate exp on DVE for a subset of non-diagonal score
# blocks: i = A8*x + B8 written as int32; the high bf16 half of each
# int32 is then ~exp(x*SCALE/W8_SCALE^2). Offload balances the scalar
# engine (exp bottleneck) against DVE headroom.
A8 = 12102203.161561485 * (1.0 / math.sqrt(1024)) / (16.0 * 16.0)
B8 = 1064986823.0   # 127*2^23 - 366393

# software-pipeline stage lags (consumers trail producers so engine wait
# queues never backpressure the sequencers)
EXP_LAG = 1
MASK_LAG = 3
AV_LAG = 6
DEBUG_NOLAG = False

_cached = {}


def r(ap):
    return ap.bitcast(F32R)


def build_program(l=L):
    """Build the SPMD Bass program (identical on all 8 cores)."""
    nc = bacc.Bacc("TRN2", target_bir_lowering=False, debug=False,
                   num_devices=NCORES)

    ic_n = l // 512        # 512-row query chunks
    nlc = N * l            # rows per core (all batches)

    xq8_nd = nc.dram_tensor("xq8_nd", [D, nlc], FP8,
                            kind="ExternalInput").ap()
    xk8_nd = nc.dram_tensor("xk8_nd", [D, nlc], FP8,
                            kind="ExternalInput").ap()
    wq8 = nc.dram_tensor("wq8", [D, FC], FP8, kind="ExternalInput").ap()
    wk8 = nc.dram_tensor("wk8", [D, FC], FP8, kind="ExternalInput").ap()
    wvt = nc.dram_tensor("wvt", [D, FC], FP8, kind="ExternalInput").ap()
    q_res = nc.dram_tensor("q_res", [nlc, FC], BF16, kind="ExternalInput").ap()
    gamma = nc.dram_tensor("gamma", [1, FC], F32, kind="ExternalInput").ap()
    beta = nc.dram_tensor("beta", [1, FC], F32, kind="ExternalInput").ap()
    out_s = nc.dram_tensor("out_s", [nlc, FC], BF16,
                           kind="ExternalOutput").ap()

    jblocks = l // 128     # 16 key blocks per batch
    nls = nlc // 128       # 64 ls blocks of res

    from contextlib import ExitStack
    with ExitStack() as stack:
        tc = stack.enter_context(tile.TileContext(nc))
        pool = {}
        for nm, bufs, space in (
                ("consts", 1, None), ("persist", 1, None), ("wt", 1, None),
                ("qtp", 2, None), ("xq", 2, None), ("xk", 2, None),
                ("xk8", 2, None),
                ("at2", 7, None), ("ati", 4, None),
                ("qin", 2, None), ("sq", 2, None),
                ("outp", 3, None), ("bnp", 1, None), ("small", 6, None),
                ("st2", 2, "PSUM"), ("av", 1, "PSUM"), ("pj", 1, "PSUM"),
                ("stat", 1, "PSUM")):
            kw = {"name": nm, "bufs": bufs}
            if space:
                kw["space"] = space
            pool[nm] = stack.enter_context(tc.tile_pool(**kw))
        consts, persist, wtp = pool["consts"], pool["persist"], pool["wt"]
        qtp, xqp, xkp = pool["qtp"], pool["xq"], pool["xk"]
        xk8p = pool["xk8"]
        at2p, qinp, sqp = pool["at2"], pool["qin"], pool["sq"]
        atip = pool["ati"]
        outp, bnp, smallp = pool["outp"], pool["bnp"], pool["small"]
        st2p, avp, pjp, statp = (pool["st2"], pool["av"], pool["pj"],
                                 pool["stat"])

        # ---------------- first activation chunk DMAs (critical path) ----
        def x_src(dram, n, ic):
            return bass.AP(
                tensor=dram.tensor,
                offset=dram.offset + n * l + ic * 512,
                ap=[[nlc, 128], [128 * nlc, 8], [1, 512]])

        def load_x(pool, dram, n, ic):
            t = pool.tile([128, 8 * 512], BF16, tag="x", name="xt")
            nc.sync.dma_start(
                t.rearrange("p (dc x) -> p dc x", dc=8), x_src(dram, n, ic))
            return t

        def x8_src(dram, n, ic):
            # d = s*256 + t*128 + p (fp8 DoubleRow pair layout)
            return bass.AP(
                tensor=dram.tensor,
                offset=dram.offset + n * l + ic * 512,
                ap=[[nlc, 128], [256 * nlc, 4], [128 * nlc, 2], [1, 512]])

        def load_x8(pool, dram, n, ic):
            t = pool.tile([128, 4 * 2 * 512], FP8, tag="x8", name="x8t")
            nc.sync.dma_start(
                t.rearrange("p (s t x) -> p s t x", s=4, t=2),
                x8_src(dram, n, ic))
            return t

        # first q-projection chain needs wq then xq: emit those two DMAs
        # first so PE can start as early as possible
        wts = {}

        def load_w8(wname, wdram):
            t = wtp.tile([128, 4 * 2 * FC], FP8, tag=wname, name=wname)
            nc.sync.dma_start(
                t.rearrange("p (s t f) -> p s t f", s=4, t=2),
                bass.AP(tensor=wdram.tensor, offset=wdram.offset,
                        ap=[[FC, 128], [256 * FC, 4], [128 * FC, 2],
                            [1, FC]]))
            wts[wname] = t

        # the very first score block only needs kt columns 0..127, so a
        # small early DMA + mini projection gets the scalar engine going
        # ~4us sooner than waiting for the full key transfer
        load_w8("wk", wk8)
        xk8_mini = xkp.tile([128, 4 * 2 * 128], FP8, tag="xm", name="xm")
        nc.sync.dma_start(
            xk8_mini.rearrange("p (s t x) -> p s t x", s=4, t=2),
            bass.AP(tensor=xk8_nd.tensor, offset=xk8_nd.offset,
                    ap=[[nlc, 128], [256 * nlc, 4], [128 * nlc, 2],
                        [1, 128]]))
        load_w8("wq", wq8)
        xq_t = load_x8(xqp, xq8_nd, 0, 0)
        xk8_t = load_x8(xk8p, xk8_nd, 0, 0)
        load_w8("wv", wvt)

        # ---------------- constants -------------------------------------
        ones_col = consts.tile([128, 1], BF16)
        nc.vector.memset(ones_col, 1.0)
        eps_sb = consts.tile([128, 1], F32)
        nc.vector.memset(eps_sb, EPS)
        gamma_sb = consts.tile([1, FC], F32)
        nc.sync.dma_start(gamma_sb, gamma)
        beta_sb = consts.tile([1, FC], F32)
        nc.sync.dma_start(beta_sb, beta)
        # lower-triangular (j <= i) mask in [j-part, i-free] layout
        tm_f = consts.tile([128, 128], F32)
        nc.vector.memset(tm_f, 1.0)
        nc.gpsimd.affine_select(
            out=tm_f, in_=tm_f,
            compare_op=mybir.AluOpType.is_ge, fill=0.0, base=0,
            pattern=[[1, 128]], channel_multiplier=-1)
        trimask = consts.tile([128, 128], BF16)
        nc.vector.tensor_copy(trimask, tm_f)
        # zero operands for the PSUM-bank-clearing matmuls (PSUM
        # start_tensor_calc marks a whole 2KB zero-region, so banks shared
        # by several accumulation groups must be cleared by one explicit
        # full-tile start matmul instead of per-group start flags)
        zlhs = consts.tile([128, 128], BF16)
        nc.vector.memset(zlhs, 0.0)
        zrhs = consts.tile([128, 260], BF16)
        nc.vector.memset(zrhs, 0.0)
        b8_sb = consts.tile([128, 1], F32)
        nc.vector.memset(b8_sb, B8)

        # ---------------- persistent SBUF -------------------------------
        # kt_sb: [feature(h*64+p), (n, j)] bf16
        kt_sb = persist.tile([128, N * l], BF16, tag="kt")
        # v_sb: [j-in-block, (n, jc, h, 65)] bf16; col 64 of each 65-group
        # is the baked ones column (softmax denominator trick)
        v_sb = persist.tile([128, N * jblocks * H2 * 65], BF16, tag="v")
        v3 = v_sb.rearrange("p (g x) -> p g x", x=65)
        nc.gpsimd.memset(v3[:, :, 64:65], 1.0)
        # res_sb: [l-in-block, (ls, f)] bf16, ls = n*16 + ic*4 + S
        res_sb = persist.tile([128, nls * FC], BF16, tag="res")

        # partition 0 row: sums; partition 64 row: sums of squares
        # (matmul outputs must start at partition 0, 32, or 64)
        stat = statp.tile([65, 512], F32, tag="stat")

        # ------------------------------------------------------------------
        # projection task machinery (filler micro-ops paced into B loops)
        # ------------------------------------------------------------------
        # rotating PSUM allocators: the interleaved fillers use the single
        # pj bank; the up-front A(0) block also rotates through the two
        # (then idle) score banks so chains overlap their copy-out
        def alloc_pj():
            return pjp.tile([128, 512], F32, tag="pj", name="pj")

        def alloc_st2_slot():
            return st2p.tile([128, 1024], F32, tag="st2", name="st2")[:, 0:512]

        rot = {"i": 0}

        def alloc_rotating():
            rot["i"] += 1
            return alloc_pj() if rot["i"] % 3 == 0 else alloc_st2_slot()

        chain_alloc = {"fn": alloc_pj}

        def qk_chain(side, n, ic, qt_tile, get_xt, col_lo=0):
            """q/k projection via fp8 DoubleRow: 4 contraction-256 steps."""
            w_use = wts["wq"] if side == "q" else wts["wk"]
            w4 = w_use.rearrange("p (s t f) -> p s t f", s=4, t=2)
            width = 512 - col_lo
            pj = {}
            alloc_fn = chain_alloc["fn"]

            def alloc():
                pj["t"] = alloc_fn()

            def mm(s0):
                x4 = get_xt().rearrange("p (s t x) -> p s t x", s=4, t=2)
                for s in (s0, s0 + 1):
                    nc.tensor.matmul(
                        pj["t"][:, 0:width], w4[:, s],
                        x4[:, s, :, col_lo:512],
                        start=(s == 0), stop=(s == 3),
                        perf_mode=mybir.MatmulPerfMode.DoubleRow)

            def copy():
                if side == "q":
                    nc.vector.tensor_copy(qt_tile[:, n * 512:(n + 1) * 512],
                                          pj["t"])
                else:
                    base = n * l + ic * 512 + col_lo
                    nc.vector.tensor_copy(kt_sb[:, base:base + width],
                                          pj["t"][:, 0:width])

            ops = [alloc]
            for s0 in range(0, 4, 2):
                ops.append(lambda s=s0: mm(s))
            ops.append(copy)
            return ops

        def wts_slice(w, dc):
            return w[:, dc * FC:(dc + 1) * FC]

        def v_chain(n, jsub, ic, get_xt):
            pj = {}
            alloc_fn = chain_alloc["fn"]
            wv4 = wts["wv"].rearrange("p (s t f) -> p s t f", s=4, t=2)

            def alloc():
                pj["t"] = alloc_fn()

            def mm(s0):
                x4 = get_xt().rearrange("p (s t x) -> p s t x", s=4, t=2)
                for s in (s0, s0 + 1):
                    nc.tensor.matmul(
                        pj["t"][:, 0:128],
                        x4[:, s, :, jsub * 128:jsub * 128 + 128],
                        wv4[:, s],
                        start=(s == 0), stop=(s == 3),
                        perf_mode=mybir.MatmulPerfMode.DoubleRow)

            def copy():
                jc = ic * 4 + jsub
                base = (n * jblocks + jc) * H2 * 65
                dst = v_sb[:, base:base + 130].rearrange(
                    "p (h x) -> p h x", h=2)[:, :, 0:64]
                src = pj["t"][:, 0:128].rearrange("p (h x) -> p h x", h=2)
                nc.vector.tensor_copy(dst, src)

            ops = [alloc]
            for s0 in range(0, 4, 2):
                ops.append(lambda s=s0: mm(s))
            ops.append(copy)
            return ops

        def build_chunk_groups(ic, qt_tile, first_x, rotate_first=False):
            """Return per-batch lists of micro-op closures for A(ic)."""
            groups = []
            xq_cur = {0: first_x[0]}
            xk8_cur = {0: first_x[1]}
            for n in range(N):
                chain_alloc["fn"] = (alloc_rotating if rotate_first and n == 0
                                     else alloc_pj)
                ops = []
                get_xq = lambda nn=n: xq_cur[nn]
                get_xk8 = lambda nn=n: xk8_cur[nn]
                # prefetch next batch's activations
                if n + 1 < N:
                    def pre(nn=n + 1):
                        xq_cur[nn] = load_x8(xqp, xq8_nd, nn, ic)
                        xk8_cur[nn] = load_x8(xk8p, xk8_nd, nn, ic)
                    ops.append(pre)
                ops += qk_chain("q", n, ic, qt_tile, get_xq)
                ops += qk_chain("k", n, ic, None, get_xk8,
                                col_lo=128 if rotate_first and n == 0 else 0)
                for jsub in range(4):
                    ops += v_chain(n, jsub, ic, get_xk8)
                groups.append(ops)
            return groups

        # ------------------------------------------------------------------
        # A(0): only batch 0's projections run up front; batches 1-3 are
        # deadline-paced into B(0)'s iterations (batch n is needed at
        # B(0) iteration 4n) so the scalar engine starts exp'ing early.
        # ------------------------------------------------------------------
        qt_next = qtp.tile([128, N * 512], BF16, tag="qt", name="qt")
        # mini projection of key block 0 (batch 0) from the early DMA, so
        # the first scores never wait on the full key transfer
        wk4 = wts["wk"].rearrange("p (s t f) -> p s t f", s=4, t=2)
        xm4 = xk8_mini.rearrange("p (s t x) -> p s t x", s=4, t=2)
        pjm = pjp.tile([128, 512], F32, tag="pj", name="pjm")
        for s in range(4):
            nc.tensor.matmul(pjm[:, 0:128], wk4[:, s], xm4[:, s],
                             start=(s == 0), stop=(s == 3),
                             perf_mode=mybir.MatmulPerfMode.DoubleRow)
        nc.vector.tensor_copy(kt_sb[:, 0:128], pjm[:, 0:128])

        groups0 = build_chunk_groups(0, qt_next, (xq_t, xk8_t),
                                     rotate_first=True)
        for op in groups0[0]:
            op()

        # ------------------------------------------------------------------
        # main loop: one software-pipelined stream over (ic, n, jc).
        # Stage schedule at step t: scores(t), exp(t-1), mask(t-2), AV(t-3)
        # so every instruction's inputs are ready when the engine decodes
        # it (the 4-deep per-engine wait queues otherwise backpressure the
        # sequencers). A(ic+1) projection micro-ops are paced in as filler.
        # ------------------------------------------------------------------
        specs = []
        for ic in range(ic_n):
            for n in range(N):
                for jc in range(4 * ic + 4):
                    specs.append((ic, n, jc))
        nspec = len(specs)
        qt_tiles = {0: qt_next}
        st2_of, at2_of, avs_of, qres_of = {}, {}, {}, {}
        # deadline-scheduled filler queue: each projection micro-op gets a
        # target iteration index; ops are emitted once the stream reaches
        # it.  Group n of chunk ic+1 is due only when B(ic+1) reaches
        # batch n, so projections spread across ~2 chunks of iterations.
        sched = []

        def schedule_group(ops, w_start, w_end):
            no = len(ops)
            span = max(1, w_end - w_start)
            for k, op in enumerate(ops):
                sched.append((w_start + (k * span) // no, op))

        def emit_due(idx):
            while sched and sched[0][0] <= idx:
                sched.pop(0)[1]()

        def stage_scores(idx):
            ic, n, jc = specs[idx]
            st2 = st2p.tile([128, 1024], F32, tag="st2", name="st2")
            st2_of[idx] = st2
            qt_cur = qt_tiles[ic]
            for h in range(H2):
                nc.tensor.matmul(
                    st2[:, h * 512:(h + 1) * 512],
                    kt_sb[h * 64:(h + 1) * 64,
                          n * l + jc * 128:n * l + jc * 128 + 128],
                    qt_cur[h * 64:(h + 1) * 64, n * 512:(n + 1) * 512],
                    start=True, stop=True)

        def offload(ic, n, jc):
            # Schraudolph-on-DVE for a spread of full (non-diagonal) blocks
            return jc - 4 * ic < 0 and jc % 4 == 2

        def stage_exp(idx):
            ic, n, jc = specs[idx]
            rr = jc - 4 * ic
            st2 = st2_of.pop(idx)
            if offload(ic, n, jc):
                ati = atip.tile([128, 1024], I32, tag="ati", name="ati")
                nc.vector.scalar_tensor_tensor(
                    out=ati, in0=st2, scalar=A8,
                    in1=bass.AP(tensor=b8_sb.tensor, offset=b8_sb.offset,
                                ap=[[b8_sb.ap[0][0], 128], [0, 1024]]),
                    op0=mybir.AluOpType.mult, op1=mybir.AluOpType.add)
                at2_of[idx] = ("i", ati)
                return
            at2 = at2p.tile([128, 1024], BF16, tag="at2", name="at2")
            at2_of[idx] = ("a", at2)
            if rr <= 0:
                nc.scalar.activation(at2, st2,
                                     mybir.ActivationFunctionType.Exp,
                                     scale=SCALE / (W8_SCALE * W8_SCALE))
            else:
                # one strided call covering both heads' causal windows
                a3 = at2.rearrange("p (h x) -> p h x", h=2)[:, :, rr * 128:]
                s3 = st2.rearrange("p (h x) -> p h x", h=2)[:, :, rr * 128:]
                nc.scalar.activation(
                    a3, s3, mybir.ActivationFunctionType.Exp,
                    scale=SCALE / (W8_SCALE * W8_SCALE))

        def stage_mask(idx):
            ic, n, jc = specs[idx]
            rr = jc - 4 * ic
            if rr < 0:
                return
            at2 = at2_of[idx][1]   # diagonal blocks are never offloaded
            for h in range(H2):
                sl = slice(h * 512 + rr * 128, h * 512 + rr * 128 + 128)
                nc.gpsimd.tensor_mul(at2[:, sl], at2[:, sl], trimask)

        def stage_av(idx):
            ic, n, jc = specs[idx]
            rr = jc - 4 * ic
            kind, at2 = at2_of.pop(idx)
            if kind == "i":
                # high bf16 half of each int32 = the approximated exp
                at2 = at2.bitcast(BF16).rearrange(
                    "p (x two) -> p x two", two=2)
            if jc == 0:
                avs_of[(ic, n)] = [avp.tile([128, 260], F32, tag=f"av{h}",
                                            name=f"av{h}")
                                   for h in range(H2)]
                for h in range(H2):
                    # clear the whole accumulator bank exactly once
                    nc.tensor.matmul(avs_of[(ic, n)][h], zlhs, zrhs,
                                     start=True, stop=True,
                                     skip_group_check=True)
            avs = avs_of[(ic, n)]
            vbase = (n * jblocks + jc) * H2 * 65
            for h in range(H2):
                for S in range(4):
                    if rr > S:
                        continue
                    base = h * 512 + S * 128
                    if kind == "i":
                        lhs = at2[:, base:base + 128, 1]
                    else:
                        lhs = at2[:, base:base + 128]
                    nc.tensor.matmul(
                        avs[h][:, S * 65:(S + 1) * 65],
                        lhs,
                        v_sb[:, vbase + h * 65:vbase + h * 65 + 65],
                        start=False, stop=(rr == S),
                        skip_group_check=True)
            if jc == 4 * ic + 3:
                enqueue_drain(ic, n)

        # drains and stats run as small deferred pieces, one per iteration,
        # so their engine dependencies are satisfied before dispatch and
        # they never block the in-order PE/DVE queues.
        drain_pending = []

        def enqueue_drain(ic, n):
            avs = avs_of.pop((ic, n))
            qres_t = qres_of.pop((ic, n))
            base512 = (n * 16 + ic * 4) * FC

            def drain_head(h):
                av3 = avs[h].rearrange("p (s x) -> p s x", x=65)
                rec = smallp.tile([128, 4], F32, tag="rec", name="rec")
                nc.vector.reciprocal(rec, av3[:, :, 64])
                for S in range(4):
                    nc.vector.scalar_tensor_tensor(
                        out=res_sb[:, base512 + S * FC + h * 64:
                                   base512 + S * FC + h * 64 + 64],
                        in0=avs[h][:, S * 65:S * 65 + 64],
                        scalar=rec[:, S:S + 1],
                        in1=qres_t[:, S * FC + h * 64:S * FC + h * 64 + 64],
                        op0=mybir.AluOpType.mult,
                        op1=mybir.AluOpType.add)

            def drain_sq():
                res_block = res_sb[:, base512:base512 + 512]
                sqt = sqp.tile([128, 512], BF16, tag="sq", name="sqt")
                nc.vector.tensor_mul(sqt, res_block, res_block)
                stats_bufs[(ic, n)] = (res_block, sqt)

            def drain_stats():
                res_block, sqt = stats_bufs.pop((ic, n))
                first = (n == 0 and ic == 0)
                last = (n == N - 1 and ic == ic_n - 1)
                nc.tensor.matmul(stat[0:1, :], ones_col, res_block,
                                 start=first, stop=last,
                                 skip_group_check=True)
                nc.tensor.matmul(stat[64:65, :], ones_col, sqt,
                                 start=first, stop=last,
                                 skip_group_check=True)

            if DEBUG_NOLAG:
                drain_head(0)
                drain_head(1)
                drain_sq()
                drain_stats()
            else:
                drain_pending.extend(
                    [lambda: drain_head(0), None,
                     lambda: (drain_head(1), drain_sq()), None, drain_stats])

        stats_bufs = {}

        def step_drain(flush=False):
            while drain_pending:
                op = drain_pending.pop(0)
                if op is None:
                    if flush:
                        continue
                    return
                op()

        # spec index of (ic, n, jc=0), for filler deadlines
        start_idx = {}
        for i, (sic, sn, sjc) in enumerate(specs):
            if sjc == 0:
                start_idx[(sic, sn)] = i

        # chunk 0's remaining groups are due just before B(0) reaches
        # that batch
        for n in range(1, N):
            schedule_group(groups0[n], start_idx[(0, n - 1)],
                           start_idx[(0, n)])

        for idx in range(nspec + max(EXP_LAG, MASK_LAG, AV_LAG)):
            if idx < nspec:
                ic, n, jc = specs[idx]
                if jc == 0 and n == 0 and ic + 1 < ic_n:
                    # build next chunk's projections; group g is paced to
                    # finish just before B(ic+1) reaches batch g
                    qt_tiles[ic + 1] = qtp.tile([128, N * 512], BF16,
                                                tag="qt", name="qt")
                    nxq = load_x8(xqp, xq8_nd, 0, ic + 1)
                    nxk8 = load_x8(xk8p, xk8_nd, 0, ic + 1)
                    groups = build_chunk_groups(ic + 1, qt_tiles[ic + 1],
                                                (nxq, nxk8))
                    w_prev = idx
                    for g in range(N):
                        w_end = start_idx[(ic + 1, g)]
                        schedule_group(groups[g], w_prev, w_end)
                        w_prev = w_end
                if jc == 0:
                    qres_t = qinp.tile([128, 4 * FC], BF16, tag="qres",
                                       name="qres")
                    nc.sync.dma_start(
                        qres_t.rearrange("p (s f) -> p s f", s=4),
                        bass.AP(tensor=q_res.tensor,
                                offset=q_res.offset + (n * l + ic * 512) * FC,
                                ap=[[FC, 128], [128 * FC, 4], [1, FC]]))
                    qres_of[(ic, n)] = qres_t
                stage_scores(idx)
            if idx - EXP_LAG >= 0 and idx - EXP_LAG < nspec:
                stage_exp(idx - EXP_LAG)
            if idx - MASK_LAG >= 0 and idx - MASK_LAG < nspec:
                stage_mask(idx - MASK_LAG)
            step_drain()
            if idx - AV_LAG >= 0 and idx - AV_LAG < nspec:
                stage_av(idx - AV_LAG)
            emit_due(idx)
        while sched:
            sched.pop(0)[1]()
        step_drain(flush=True)

        # ------------------------------------------------------------------
        # batch-norm: fold partial sums, compute gamma', beta', apply
        # ------------------------------------------------------------------
        # fold the 4 ls-group partials to [1, FC]: one strided reduction
        # per stats row, reading PSUM directly (feature-major view so the
        # 4 group values of each feature are the innermost axis)
        def stat_fold_view(prow):
            return bass.AP(tensor=stat.tensor,
                           offset=stat.offset + prow * stat.ap[0][0],
                           ap=[[stat.ap[0][0], 1], [1, FC], [FC, 4]])

        sumf = bnp.tile([1, FC], F32, tag="sumf", name="sumf")
        nc.vector.reduce_sum(sumf, stat_fold_view(0),
                             axis=mybir.AxisListType.X)
        sqf = bnp.tile([1, FC], F32, tag="sqf", name="sqf")
        nc.vector.reduce_sum(sqf, stat_fold_view(64),
                             axis=mybir.AxisListType.X)

        inv = 1.0 / NL
        mean = bnp.tile([1, FC], F32, tag="mean", name="mean")
        nc.vector.tensor_scalar_mul(mean, sumf, inv)
        musq = bnp.tile([1, FC], F32, tag="musq", name="musq")   # mean^2
        nc.vector.tensor_mul(musq, mean, mean)
        var = bnp.tile([1, FC], F32, tag="var", name="var")
        nc.vector.scalar_tensor_tensor(
            out=var, in0=sqf, scalar=inv, in1=musq,
            op0=mybir.AluOpType.mult, op1=mybir.AluOpType.subtract)
        std = bnp.tile([1, FC], F32, tag="std", name="std")
        nc.scalar.activation(std, var, mybir.ActivationFunctionType.Sqrt,
                             bias=eps_sb[0:1, :])
        rstd = bnp.tile([1, FC], F32, tag="rstd", name="rstd")
        nc.vector.reciprocal(rstd, std)
        gp = bnp.tile([1, FC], F32, tag="gp", name="gp")
        nc.vector.tensor_mul(gp, gamma_sb, rstd)
        mgp = bnp.tile([1, FC], F32, tag="mgp", name="mgp")
        nc.vector.tensor_mul(mgp, mean, gp)
        bp = bnp.tile([1, FC], F32, tag="bp", name="bp")
        nc.vector.tensor_sub(bp, beta_sb, mgp)
        gp16 = bnp.tile([1, FC], BF16, tag="gp16", name="gp16")
        nc.vector.tensor_copy(gp16, gp)
        bp16 = bnp.tile([1, FC], BF16, tag="bp16", name="bp16")
        nc.vector.tensor_copy(bp16, bp)

        gbc = bnp.tile([128, FC], BF16, tag="gbc", name="gbc")
        nc.gpsimd.partition_broadcast(gbc, gp16)
        bbc = bnp.tile([128, FC], BF16, tag="bbc", name="bbc")
        nc.gpsimd.partition_broadcast(bbc, bp16)

        def rep4(t):
            return bass.AP(tensor=t.tensor, offset=t.offset,
                           ap=[[t.ap[0][0], 128], [0, 4], [1, FC]])

        gbc4 = bnp.tile([128, 512], BF16, tag="gbc4", name="gbc4")
        nc.vector.tensor_copy(gbc4, rep4(gbc))
        bbc4 = bnp.tile([128, 512], BF16, tag="bbc4", name="bbc4")
        nc.vector.tensor_copy(bbc4, rep4(bbc))

        for n in range(N):
            for ic in range(ic_n):
                base512 = (n * 16 + ic * 4) * FC
                t1 = outp.tile([128, 512], BF16, tag="t1", name="t1")
                nc.vector.tensor_mul(t1, res_sb[:, base512:base512 + 512],
                                     gbc4)
                t2 = outp.tile([128, 512], BF16, tag="t2", name="t2")
                nc.vector.tensor_add(t2, t1, bbc4)
                nc.sync.dma_start(
                    bass.AP(tensor=out_s.tensor,
                            offset=out_s.offset + (n * l + ic * 512) * FC,
                            ap=[[FC, 128], [128 * FC, 4], [1, FC]]),
                    t2.rearrange("p (s f) -> p s f", s=4))

    nc.compile()
    return nc


def get_runner(nc):
    """Build (once) a cached jitted SPMD executor for the Bass program."""
    if "runner" in _cached:
        return _cached["runner"]

    import jax
    from jax.experimental.shard_map import shard_map
    from jax.sharding import Mesh, PartitionSpec
    from concourse import bass2jax

    bass2jax.install_neuronx_cc_hook()

    partition_name = (nc.partition_id_tensor.name
                      if nc.partition_id_tensor else None)
    in_names, out_names, out_avals, zero_outs = [], [], [], []
    for alloc in nc.m.functions[0].allocations:
        if not isinstance(alloc, mybir.MemoryLocationSet):
            continue
        name = alloc.memorylocations[0].name
        if alloc.kind == "ExternalInput":
            if name != partition_name:
                in_names.append(name)
        elif alloc.kind == "ExternalOutput":
            shape = tuple(alloc.tensor_shape)
            dtype = mybir.dt.np(alloc.dtype)
            out_names.append(name)
            out_avals.append(jax.core.ShapedArray(shape, dtype))
            zero_outs.append(np.zeros(shape, dtype))
    n_params = len(in_names)
    n_outs = len(out_avals)
    all_names = in_names + out_names
    if partition_name is not None:
        all_names = all_names + [partition_name]

    def _body(*args):
        operands = list(args)
        if partition_name is not None:
            operands.append(bass2jax.partition_id_tensor())
        outs = bass2jax._bass_exec_p.bind(
            *operands,
            out_avals=tuple(out_avals),
            in_names=tuple(all_names),
            out_names=tuple(out_names),
            lowering_input_output_aliases=(),
            sim_require_finite=True,
            sim_require_nnan=True,
            nc=nc,
        )
        return tuple(outs)

    devices = jax.devices()[:NCORES]
    mesh = Mesh(np.asarray(devices), ("core",))
    in_specs = (PartitionSpec("core"),) * (n_params + n_outs)
    out_specs = (PartitionSpec("core"),) * n_outs
    donate = tuple(range(n_params, n_params + n_outs))
    sharded = jax.jit(
        shard_map(_body, mesh=mesh, in_specs=in_specs, out_specs=out_specs,
                  check_rep=False),
        donate_argnums=donate, keep_unused=True)

    def run_np(in_maps):
        concat_in = [
            np.concatenate([np.asarray(in_maps[c][nm]) for c in range(NCORES)],
                           axis=0)
            for nm in in_names]
        concat_zeros = [np.zeros((NCORES * z.shape[0], *z.shape[1:]), z.dtype)
                        for z in zero_outs]
        out_arrs = sharded(*concat_in, *concat_zeros)
        return [
            {nm: np.asarray(out_arrs[i]).reshape(
                NCORES, *out_avals[i].shape)[c]
             for i, nm in enumerate(out_names)}
            for c in range(NCORES)]

    _cached["runner"] = (run_np, sharded, in_names, out_names, out_avals,
                         zero_outs, mesh)
    return _cached["runner"]


def make_in_maps(inputs, l):
    query = np.asarray(inputs["query"], dtype=np.float32)
    key = np.asarray(inputs["key"], dtype=np.float32)
    Wq = np.asarray(inputs["Wq"], dtype=np.float32)
    Wk = np.asarray(inputs["Wk"], dtype=np.float32)
    Wv = np.asarray(inputs["Wv"], dtype=np.float32)
    gamma = np.asarray(inputs["gamma"], dtype=np.float32)
    beta = np.asarray(inputs["beta"], dtype=np.float32)

    n = query.shape[0]
    qf = query.reshape(n * l, D)
    kf = key.reshape(n * l, D)
    xq8 = np.ascontiguousarray(qf.T.astype(FP8_NP))
    xk8 = np.ascontiguousarray(kf.T.astype(FP8_NP))

    in_maps = []
    for c in range(NCORES):
        sl = slice(c * FC, (c + 1) * FC)
        in_maps.append({
            "xq8_nd": xq8,
            "xk8_nd": xk8,
            "wq8": np.ascontiguousarray(
                (Wq[sl].T * W8_SCALE).astype(FP8_NP)),
            "wk8": np.ascontiguousarray(
                (Wk[sl].T * W8_SCALE).astype(FP8_NP)),
            "wvt": np.ascontiguousarray(Wv[sl].T.astype(FP8_NP)),
            "q_res": np.ascontiguousarray(qf[:, sl].astype(BF16_NP)),
            "gamma": np.ascontiguousarray(gamma[sl].reshape(1, FC)),
            "beta": np.ascontiguousarray(beta[sl].reshape(1, FC)),
        })
    return in_maps


def kernel(**inputs):
    l = np.asarray(inputs["query"]).shape[1]
    if "nc" not in _cached or _cached.get("l") != l:
        _cached["nc"] = build_program(l)
        _cached["l"] = l
    nc = _cached["nc"]

    in_maps = make_in_maps(inputs, l)
    run_np = get_runner(nc)[0]
    results = run_np(in_maps)

    n = np.asarray(inputs["query"]).shape[0]
    out = np.zeros((n, l, D), dtype=np.float32)
    for c in range(NCORES):
        sl = slice(c * FC, (c + 1) * FC)
        out[:, :, sl] = results[c]["out_s"].reshape(n, l, FC).astype(
            np.float32)
    return out
